# revision 6
# baseline (speedup 1.0000x reference)
"""VQ codebook (DKVB) kernel for Trainium2, sharded over 8 NeuronCores.

Problem: embeddings [8, 2048, 4, 4] -> tokens x [128, 256, 8]; per codebook c
(256 of them), find nearest code among 4096 (euclidean), gather values row.

Strategy: shard the 256 codebooks across 8 cores (32 per core).
Score s[t,k] = x_t . k - |k|^2/2 (argmax s == argmin dist), computed as a
bf16 Dekker split folded into ONE matmul with contraction dim 34:
    lhsT rows = [xh(8); 1; xh(8); 1; xl(8); xl(8)]
    rhs  rows = [kh(8); -sh; kl(8); -sl; kh(8); kl(8)]

The PE runs at a fixed 512cols/427ns (mid p-state is the steady-state ceiling
on this stack), so the PE is the pacer: 8 matmuls/cb = 3.4us/cb = 109us.
Everything else is sized to hide under that:
  - Act: 2x copy [T,1024] PSUM->SBUF per cb (2.3us)
  - DVE: seed+continuation pair-scan, 2 scores/cycle (2.5us)
  - keysT DMA: codebooks are packed 2-per-free-range at partition offsets
    {0,64} (DMA cost on this stack is free-bytes-per-partition only), and
    loaded 4 codebooks per DMA on the SP queue (~50us total, hidden).

Argmax: custom two-source DVE op (ARGMAX2) pairs the PSUM stream (codes
2048+j, PSUM port) with the SBUF stream (codes j, evacuated by ScalarE):
    v = 2*j + (e_sbuf >= e_psum)
accum-max(v) is the encoded argmax; values are host-permuted so v indexes
them directly. The scan state lives in DVE stage flops which persist across
instructions, so the 2048-pair scan is seed (ARGMAX2) + continuation
(ARGMAX2C), letting PSUM quarters release mid-scan.
"""

import numpy as np
import ml_dtypes

import concourse.bass as bass
import concourse.tile as tile
from concourse import bacc, mybir
from concourse.bass_utils import run_bass_kernel_spmd

B, D, H, W = 8, 2048, 4, 4
C, K, d = 256, 4096, 8
NCORES = 8
CBC = C // NCORES          # 32 codebooks per core
T = B * H * W              # 128 tokens
KA = 34                    # Dekker-augmented contraction dim
HK = K // 2                # 2048
QK = K // 4                # 1024

F32 = mybir.dt.float32
BF16 = mybir.dt.bfloat16
U32 = mybir.dt.uint32

_FLT_MAX = np.float32(3.4028235e38)

CB_PER_DMA = 4             # 2 free-slots x {0,64} partition offsets


def _build_argmax2_spec():
    from concourse.dve_spec import (
        Spec, Src0, Src1, MaxNeg, Zero, C0, AluOp, Bin, Scan, select, maxx,
        scan,
    )

    pm = Bin(AluOp.MAX, Src0, Src1)
    nb = Bin(AluOp.IS_LE, Src0, Src1)      # 1 -> Src1 (sbuf, low codes) wins
    idx2 = Scan(AluOp.ADD, C0, init=Bin(AluOp.SUBTRACT, Zero, C0))  # 0,2,4,..
    v = Bin(AluOp.ADD, idx2, nb)
    g = scan(AluOp.MAX, pm)
    t = Bin(AluOp.IS_GE, pm, g)
    body = select(t, v, MaxNeg)

    def _ref(in0, in1, s0, s1, imm2):
        e0 = in0.astype(np.float32)
        e1 = in1.astype(np.float32)
        pm = np.maximum(e0, e1)
        nb = (e0 <= e1).astype(np.float32)
        r = np.maximum.accumulate(pm, axis=-1)
        j = np.broadcast_to(
            np.arange(pm.shape[-1], dtype=np.float32) * np.float32(s0), pm.shape
        )
        vv = j + nb
        bodyv = np.where(pm >= r, vv, -_FLT_MAX).astype(np.float32)
        acc = bodyv.reshape(bodyv.shape[0], -1).max(axis=-1, keepdims=True)
        return bodyv, acc

    return Spec(body=body, accum=maxx, reference=_ref)


def _register_ops():
    """Register ARGMAX2 (seeded) and ARGMAX2C (continuation). Idempotent."""
    from concourse import dve_ops
    from concourse.dve_spec import lower
    from concourse.dve_uop import DveOpSpec
    from dataclasses import dataclass

    have = {op.name: op for op in dve_ops.OPS}
    if "ARGMAX2_ANT" in have and "ARGMAX2C_ANT" in have:
        return have["ARGMAX2_ANT"], have["ARGMAX2C_ANT"]

    spec = _build_argmax2_spec()

    @dataclass(frozen=True)
    class ContDveOp(dve_ops.DveOp):
        """Continuation variant: steady-state uop only — inherits the scan /
        counter / accumulator state left by the previous DVE instruction."""

        def compile(self, ver):
            key = (self.name, ver)
            cached = dve_ops._COMPILE_CACHE.get(key)
            if cached is not None:
                return cached
            full = lower(self.spec, ver=ver)
            result = DveOpSpec(
                name=self.name,
                opcode=dve_ops.get_dve_sub_opcode(self.name),
                uops=[full[-1]],
                rd1_en=True,
            )
            got = result.sha(ver)
            if self.uops_sha.get(ver) != got:
                raise ValueError(f"{self.name}: sha drift {got}")
            dve_ops._COMPILE_CACHE[key] = result
            return result

    ops = []
    for name, cls, pick in (
        ("ARGMAX2_ANT", dve_ops.DveOp, None),
        ("ARGMAX2C_ANT", ContDveOp, -1),
    ):
        opcode = dve_ops._CUSTOM_DVE_ROW_BASE + len(dve_ops.OPS)
        shas = {}
        for ver in ("v3", "v4"):
            uops = lower(spec, ver=ver)
            if pick is not None:
                uops = [uops[pick]]
            s = DveOpSpec(name=name, opcode=opcode, uops=uops, rd1_en=True)
            shas[ver] = s.sha(ver)
        op = cls(name, spec, subdim=False, uops_sha=shas)
        dve_ops.OPS.append(op)
        dve_ops._SUB_OPCODE_FOR_NAME[op.name] = opcode
        dve_ops.CUSTOM_DVE_SPECS[op.name] = spec
        ops.append(op)
    return ops[0], ops[1]


def build_program():
    amax_op, amax_cont_op = _register_ops()
    nc = bacc.Bacc(trn_type="TRN2", num_devices=NCORES)

    NDMA = CBC // CB_PER_DMA   # 8 keysT blocks of 4 codebooks each

    # x rows duplicated at partitions 0-33 and 64-97; slot = cb//2
    xT = nc.dram_tensor("xT", [128, CBC // 2, T], BF16, kind="ExternalInput")
    # keysT packed: dma block i holds cb 4i..4i+3 as [128, 2, 4096]
    # (partitions 0-33 = even cb of the pair, 64-97 = odd cb)
    keysT = nc.dram_tensor("keysT", [NDMA, 128, 2, K], BF16,
                           kind="ExternalInput")
    vals = nc.dram_tensor("vals", [CBC * K, d], F32, kind="ExternalInput")
    out = nc.dram_tensor("out", [T, CBC * d], F32, kind="ExternalOutput")
    GB = 2                     # gather/cast batch (codebooks)

    with tile.TileContext(nc) as tc:
        with (
            tc.tile_pool(name="xsb", bufs=1) as x_pool,
            tc.tile_pool(name="kT", bufs=3) as kT_pool,
            tc.tile_pool(name="evac", bufs=4) as ev_pool,
            tc.tile_pool(name="scratch", bufs=1) as scr_pool,
            tc.tile_pool(name="persist", bufs=1) as persist_pool,
            tc.tile_pool(name="psum", bufs=1, space="PSUM") as psum_pool,
        ):
            x_sb = x_pool.tile([128, CBC // 2, T], BF16)

            idxf = persist_pool.tile([T, CBC], F32)
            idx_u = persist_pool.tile([T, CBC], U32)
            g = persist_pool.tile([T, CBC, d], F32)
            # single scratch shared by every ARGMAX2 op: the WAW chain forces
            # the vector engine to execute all pair-scans in emission order,
            # so nothing can slip between a seed op and its continuation
            scr = scr_pool.tile([T, QK], F32)

            # 4 persistent PSUM quarter-pairs: Pc1,Pc2 = evacuated
            # (codes 0..2047), Pd1,Pd2 = scanned in place (codes 2048..4095)
            Pc1 = psum_pool.tile([T, QK], F32, name="pc1")
            Pc2 = psum_pool.tile([T, QK], F32, name="pc2")
            Pd1 = psum_pool.tile([T, QK], F32, name="pd1")
            Pd2 = psum_pool.tile([T, QK], F32, name="pd2")

            kTs = [None] * NDMA

            def load_kT(i):
                kT = kT_pool.tile([128, 2, K], BF16, name="kt")
                kTs[i] = kT
                if i == 0:
                    # staircase: per-codebook pieces so cb0 is ready ASAP
                    for s2 in range(2):
                        for b2 in range(2):
                            o2 = 64 * b2
                            nc.sync.dma_start(
                                kT[o2:o2 + KA, s2],
                                keysT.ap()[i, o2:o2 + KA, s2],
                            )
                            if s2 == 0 and b2 == 0:
                                nc.sync.dma_start(x_sb[:], xT.ap())
                else:
                    nc.sync.dma_start(kT[:], keysT.ap()[i])

            def do_cb(cb):
                kT = kTs[cb // CB_PER_DMA]
                slot = (cb % CB_PER_DMA) // 2
                base = 64 * (cb % 2)
                lhsT = x_sb[base:base + KA, cb // 2]
                rhs = kT[base:base + KA, slot]

                # copy halves first: codes 0..1023 then 1024..2047
                nc.tensor.matmul(Pc1[:, 0:512], lhsT=lhsT, rhs=rhs[:, 0:512],
                                 start=True, stop=True)
                nc.tensor.matmul(Pc1[:, 512:1024], lhsT=lhsT,
                                 rhs=rhs[:, 512:1024], start=True, stop=True)
                ev1 = ev_pool.tile([T, QK], F32, name="ev")
                nc.scalar.copy(ev1[:], Pc1[:])
                nc.tensor.matmul(Pc2[:, 0:512], lhsT=lhsT,
                                 rhs=rhs[:, 1024:1536], start=True, stop=True)
                nc.tensor.matmul(Pc2[:, 512:1024], lhsT=lhsT,
                                 rhs=rhs[:, 1536:2048], start=True, stop=True)
                ev2 = ev_pool.tile([T, QK], F32, name="ev")
                nc.scalar.copy(ev2[:], Pc2[:])
                # direct halves: codes 2048..3071 then 3072..4095
                nc.tensor.matmul(Pd1[:, 0:512], lhsT=lhsT,
                                 rhs=rhs[:, 2048:2560], start=True, stop=True)
                nc.tensor.matmul(Pd1[:, 512:1024], lhsT=lhsT,
                                 rhs=rhs[:, 2560:3072], start=True, stop=True)
                nc.vector._custom_dve(
                    amax_op, out=scr[:], in0=Pd1[:], in1=ev1[:], s0=2.0,
                )
                nc.tensor.matmul(Pd2[:, 0:512], lhsT=lhsT,
                                 rhs=rhs[:, 3072:3584], start=True, stop=True)
                nc.tensor.matmul(Pd2[:, 512:1024], lhsT=lhsT,
                                 rhs=rhs[:, 3584:4096], start=True, stop=True)
                nc.vector._custom_dve(
                    amax_cont_op, out=scr[:], in0=Pd2[:], in1=ev2[:], s0=2.0,
                    accum_out=idxf[:, cb:cb + 1],
                )
                # batched u32 cast + value gathers
                if cb % GB == GB - 1:
                    c0 = cb - GB + 1
                    nc.gpsimd.dma_start(idx_u[:, c0:cb + 1],
                                        idxf[:, c0:cb + 1])
                    for cg in range(c0, cb + 1):
                        nc.gpsimd.indirect_dma_start(
                            out=g[:, cg],
                            out_offset=None,
                            in_=vals.ap(),
                            in_offset=bass.IndirectOffsetOnAxis(
                                ap=idx_u[:, cg:cg + 1], axis=0
                            ),
                            element_offset=cg * K * d,
                            bounds_check=K - 1,
                            oob_is_err=False,
                        )

            # staircase prefetch: keep >=2 blocks in flight ahead of compute
            load_kT(0)
            load_kT(1)
            for i in range(CBC):
                if i % CB_PER_DMA == 0:
                    nxt = i // CB_PER_DMA + 2
                    if nxt < NDMA:
                        load_kT(nxt)
                do_cb(i)
                if i == CBC - 9:
                    # early partial store: first 24 codebooks' gathers done
                    nc.sync.dma_start(
                        out.ap()[:, 0:(CBC - 8) * d],
                        g[:, 0:CBC - 8].rearrange("t c dd -> t (c dd)"),
                    )

            nc.sync.dma_start(
                out.ap()[:, (CBC - 8) * d:],
                g[:, CBC - 8:].rearrange("t c dd -> t (c dd)"),
            )

    nc.compile()
    return nc


def _bf16_split(a: np.ndarray):
    """Dekker split: a ~= hi + lo with hi, lo exactly representable in bf16."""
    hi = a.astype(ml_dtypes.bfloat16)
    lo = (a - hi.astype(np.float32)).astype(ml_dtypes.bfloat16)
    return hi, lo


def make_core_inputs(embeddings: np.ndarray, keys: np.ndarray, values: np.ndarray):
    """Host-side shard prep. Returns list of input dicts, one per core."""
    NDMA = CBC // CB_PER_DMA
    # tokens: [B, D, H, W] -> [B*N, C, d]
    x = embeddings.reshape(B, D, H * W).transpose(0, 2, 1).reshape(T, C, d)
    xh, xl = _bf16_split(np.ascontiguousarray(x))
    # lhsT rows: [xh(8); 1; xh(8); 1; xl(8); xl(8)] -> [C, 34, T]
    rows = np.empty((C, KA, T), dtype=ml_dtypes.bfloat16)
    rows[:, 0:8] = xh.transpose(1, 2, 0)
    rows[:, 8] = 1.0
    rows[:, 9:17] = rows[:, 0:8]
    rows[:, 17] = 1.0
    rows[:, 18:26] = xl.transpose(1, 2, 0)
    rows[:, 26:34] = rows[:, 18:26]

    kh, kl = _bf16_split(keys)
    s = 0.5 * np.einsum("ckd,ckd->ck", keys, keys)
    sh, sl = _bf16_split(s)
    keysT = np.empty((C, KA, K), dtype=ml_dtypes.bfloat16)
    keysT[:, 0:8] = kh.transpose(0, 2, 1)
    keysT[:, 8] = -sh
    keysT[:, 9:17] = kl.transpose(0, 2, 1)
    keysT[:, 17] = -sl
    keysT[:, 18:26] = keysT[:, 0:8]
    keysT[:, 26:34] = keysT[:, 9:17]

    # values permuted so the encoded argmax v = 2*j + nb indexes directly:
    # v even -> code 2048 + v/2 (psum half), v odd -> code (v-1)/2 (evac half)
    v = np.arange(K)
    perm = np.where(v % 2 == 0, HK + v // 2, v // 2)
    vals_perm = values[:, perm, :]

    in_maps = []
    for i in range(NCORES):
        sl_ = slice(i * CBC, (i + 1) * CBC)
        rows_c = rows[sl_]                     # [CBC, 34, T]
        keysT_c = keysT[sl_]                   # [CBC, 34, K]

        xT = np.zeros((128, CBC // 2, T), dtype=ml_dtypes.bfloat16)
        xT[0:KA] = rows_c[0::2].transpose(1, 0, 2)
        xT[64:64 + KA] = rows_c[1::2].transpose(1, 0, 2)

        kt = np.zeros((NDMA, 128, 2, K), dtype=ml_dtypes.bfloat16)
        for blk in range(NDMA):
            for s2 in range(2):
                kt[blk, 0:KA, s2] = keysT_c[blk * 4 + 2 * s2]
                kt[blk, 64:64 + KA, s2] = keysT_c[blk * 4 + 2 * s2 + 1]

        in_maps.append({
            "xT": xT,
            "keysT": kt,
            "vals": np.ascontiguousarray(
                vals_perm[sl_].reshape(CBC * K, d).astype(np.float32)
            ),
        })
    return in_maps


def assemble_output(results: list) -> np.ndarray:
    """results[i]["out"] is [T, CBC*d] for core i; -> [B, D, H, W]."""
    mem = np.concatenate(
        [np.asarray(r["out"]).reshape(T, CBC * d) for r in results], axis=1
    )  # [T, C*d] == [B*N, D]
    return (
        mem.reshape(B, H * W, D).transpose(0, 2, 1).reshape(B, D, H, W)
    ).astype(np.float32)


_CACHED_NC = None


def kernel(embeddings, keys, values):
    global _CACHED_NC
    embeddings = np.asarray(embeddings, dtype=np.float32)
    keys = np.asarray(keys, dtype=np.float32)
    values = np.asarray(values, dtype=np.float32)
    if _CACHED_NC is None:
        _CACHED_NC = build_program()
    in_maps = make_core_inputs(embeddings, keys, values)
    res = run_bass_kernel_spmd(_CACHED_NC, in_maps, list(range(NCORES)))
    return assemble_output(res.results)


if __name__ == "__main__":
    rng = np.random.default_rng(0)
    emb = rng.standard_normal((B, D, H, W), dtype=np.float32)
    ks = rng.standard_normal((C, K, d), dtype=np.float32)
    vs = rng.standard_normal((C, K, d), dtype=np.float32)
    out = kernel(emb, ks, vs)
    print("out", out.shape, out.dtype, out.ravel()[:4])


# revision 8
# speedup vs baseline: 1.0687x; 1.0687x over previous
"""VQ codebook (DKVB) kernel for Trainium2, sharded over 8 NeuronCores.

Problem: embeddings [8, 2048, 4, 4] -> tokens x [128, 256, 8]; per codebook c
(256 of them), find nearest code among 4096 (euclidean), gather values row.

Strategy: shard the 256 codebooks across 8 cores (32 per core).
Score s[t,k] = x_t . k - |k|^2/2 (argmax s == argmin dist), computed as a
bf16 Dekker split folded into ONE matmul with contraction dim 34:
    lhsT rows = [xh(8); 1; xh(8); 1; xl(8); xl(8)]
    rhs  rows = [kh(8); -sh; kl(8); -sl; kh(8); kl(8)]

The PE runs at a fixed 512cols/427ns (mid p-state is the steady-state ceiling
on this stack), so the PE is the pacer: 8 matmuls/cb = 3.4us/cb = 109us.
Everything else is sized to hide under that:
  - Act: 2x copy [T,1024] PSUM->SBUF per cb (2.3us)
  - DVE: seed+continuation pair-scan, 2 scores/cycle (2.5us)
  - keysT DMA: codebooks are packed 2-per-free-range at partition offsets
    {0,64} (DMA cost on this stack is free-bytes-per-partition only), and
    loaded 4 codebooks per DMA on the SP queue (~50us total, hidden).

Argmax: custom two-source DVE op (ARGMAX2) pairs the PSUM stream (codes
2048+j, PSUM port) with the SBUF stream (codes j, evacuated by ScalarE):
    v = 2*j + (e_sbuf >= e_psum)
accum-max(v) is the encoded argmax; values are host-permuted so v indexes
them directly. The scan state lives in DVE stage flops which persist across
instructions, so the 2048-pair scan is seed (ARGMAX2) + continuation
(ARGMAX2C), letting PSUM quarters release mid-scan.
"""

import numpy as np
import ml_dtypes

import concourse.bass as bass
import concourse.tile as tile
from concourse import bacc, mybir
from concourse.bass_utils import run_bass_kernel_spmd

B, D, H, W = 8, 2048, 4, 4
C, K, d = 256, 4096, 8
NCORES = 8
CBC = C // NCORES          # 32 codebooks per core
T = B * H * W              # 128 tokens
KA = 34                    # Dekker-augmented contraction dim
HK = K // 2                # 2048
QK = K // 4                # 1024

F32 = mybir.dt.float32
BF16 = mybir.dt.bfloat16
U32 = mybir.dt.uint32

_FLT_MAX = np.float32(3.4028235e38)

CB_PER_DMA = 4             # 2 free-slots x {0,64} partition offsets


def _build_argmax2_spec():
    from concourse.dve_spec import (
        Spec, Src0, Src1, MaxNeg, Zero, C0, AluOp, Bin, Scan, select, maxx,
        scan,
    )

    pm = Bin(AluOp.MAX, Src0, Src1)
    nb = Bin(AluOp.IS_LE, Src0, Src1)      # 1 -> Src1 (sbuf, low codes) wins
    idx2 = Scan(AluOp.ADD, C0, init=Bin(AluOp.SUBTRACT, Zero, C0))  # 0,2,4,..
    v = Bin(AluOp.ADD, idx2, nb)
    g = scan(AluOp.MAX, pm)
    t = Bin(AluOp.IS_GE, pm, g)
    body = select(t, v, MaxNeg)

    def _ref(in0, in1, s0, s1, imm2):
        e0 = in0.astype(np.float32)
        e1 = in1.astype(np.float32)
        pm = np.maximum(e0, e1)
        nb = (e0 <= e1).astype(np.float32)
        r = np.maximum.accumulate(pm, axis=-1)
        j = np.broadcast_to(
            np.arange(pm.shape[-1], dtype=np.float32) * np.float32(s0), pm.shape
        )
        vv = j + nb
        bodyv = np.where(pm >= r, vv, -_FLT_MAX).astype(np.float32)
        acc = bodyv.reshape(bodyv.shape[0], -1).max(axis=-1, keepdims=True)
        return bodyv, acc

    return Spec(body=body, accum=maxx, reference=_ref)


def _register_ops():
    """Register ARGMAX2 (seeded) and ARGMAX2C (continuation). Idempotent."""
    from concourse import dve_ops
    from concourse.dve_spec import lower
    from concourse.dve_uop import DveOpSpec
    from dataclasses import dataclass

    have = {op.name: op for op in dve_ops.OPS}
    if "ARGMAX2_ANT" in have and "ARGMAX2C_ANT" in have:
        return have["ARGMAX2_ANT"], have["ARGMAX2C_ANT"]

    spec = _build_argmax2_spec()

    @dataclass(frozen=True)
    class ContDveOp(dve_ops.DveOp):
        """Continuation variant: steady-state uop only — inherits the scan /
        counter / accumulator state left by the previous DVE instruction."""

        def compile(self, ver):
            key = (self.name, ver)
            cached = dve_ops._COMPILE_CACHE.get(key)
            if cached is not None:
                return cached
            full = lower(self.spec, ver=ver)
            result = DveOpSpec(
                name=self.name,
                opcode=dve_ops.get_dve_sub_opcode(self.name),
                uops=[full[-1]],
                rd1_en=True,
            )
            got = result.sha(ver)
            if self.uops_sha.get(ver) != got:
                raise ValueError(f"{self.name}: sha drift {got}")
            dve_ops._COMPILE_CACHE[key] = result
            return result

    ops = []
    for name, cls, pick in (
        ("ARGMAX2_ANT", dve_ops.DveOp, None),
        ("ARGMAX2C_ANT", ContDveOp, -1),
    ):
        opcode = dve_ops._CUSTOM_DVE_ROW_BASE + len(dve_ops.OPS)
        shas = {}
        for ver in ("v3", "v4"):
            uops = lower(spec, ver=ver)
            if pick is not None:
                uops = [uops[pick]]
            s = DveOpSpec(name=name, opcode=opcode, uops=uops, rd1_en=True)
            shas[ver] = s.sha(ver)
        op = cls(name, spec, subdim=False, uops_sha=shas)
        dve_ops.OPS.append(op)
        dve_ops._SUB_OPCODE_FOR_NAME[op.name] = opcode
        dve_ops.CUSTOM_DVE_SPECS[op.name] = spec
        ops.append(op)
    return ops[0], ops[1]


def build_program():
    amax_op, amax_cont_op = _register_ops()
    nc = bacc.Bacc(trn_type="TRN2", num_devices=NCORES)

    NDMA = CBC // CB_PER_DMA   # 8 keysT blocks of 4 codebooks each

    # x rows duplicated at partitions 0-33 and 64-97; slot = cb//2
    xT = nc.dram_tensor("xT", [128, CBC // 2, T], BF16, kind="ExternalInput")
    # keysT packed: dma block i holds cb 4i..4i+3 as [128, 2, 4096]
    # (partitions 0-33 = even cb of the pair, 64-97 = odd cb)
    keysT = nc.dram_tensor("keysT", [NDMA, 128, 2, K], BF16,
                           kind="ExternalInput")
    vals = nc.dram_tensor("vals", [CBC * K, d], F32, kind="ExternalInput")
    out = nc.dram_tensor("out", [T, CBC * d], F32, kind="ExternalOutput")
    GB = 2                     # gather/cast batch (codebooks)

    with tile.TileContext(nc) as tc:
        with (
            tc.tile_pool(name="xsb", bufs=1) as x_pool,
            tc.tile_pool(name="kT", bufs=3) as kT_pool,
            tc.tile_pool(name="evac", bufs=4) as ev_pool,
            tc.tile_pool(name="scratch", bufs=1) as scr_pool,
            tc.tile_pool(name="persist", bufs=1) as persist_pool,
            tc.tile_pool(name="psum", bufs=1, space="PSUM") as psum_pool,
        ):
            x_sb = x_pool.tile([128, CBC // 2, T], BF16)

            idxf = persist_pool.tile([T, CBC], F32)
            idx_u = persist_pool.tile([T, CBC], U32)
            g = persist_pool.tile([T, CBC, d], F32)
            # single scratch shared by every ARGMAX2 op: the WAW chain forces
            # the vector engine to execute all pair-scans in emission order,
            # so nothing can slip between a seed op and its continuation
            scr = scr_pool.tile([T, QK], F32)

            # PSUM: copy half 1 split into two 1-bank tiles (shortens the
            # copy1 -> next-cb-matmul WAR arc, the critical cycle)
            Pc1a = psum_pool.tile([T, 512], F32, name="pc1a")
            Pc1b = psum_pool.tile([T, 512], F32, name="pc1b")
            Pc2 = psum_pool.tile([T, QK], F32, name="pc2")
            Pd1 = psum_pool.tile([T, QK], F32, name="pd1")
            Pd2 = psum_pool.tile([T, QK], F32, name="pd2")

            kTs = [None] * NDMA

            def load_kT(i):
                kT = kT_pool.tile([128, 2, K], BF16, name="kt")
                kTs[i] = kT
                if i == 0:
                    # staircase: slot pieces so cb0/1 are ready ASAP
                    nc.sync.dma_start(kT[:, 0], keysT.ap()[i, :, 0])
                    nc.sync.dma_start(kT[:, 1], keysT.ap()[i, :, 1])
                else:
                    nc.sync.dma_start(kT[:], keysT.ap()[i])

            def do_cb(cb):
                kT = kTs[cb // CB_PER_DMA]
                slot = (cb % CB_PER_DMA) // 2
                base = 64 * (cb % 2)
                lhsT = x_sb[base:base + KA, cb // 2]
                rhs = kT[base:base + KA, slot]

                # copy-half 1: codes 0..1023 (two 1-bank tiles)
                nc.tensor.matmul(Pc1a[:], lhsT=lhsT, rhs=rhs[:, 0:512],
                                 start=True, stop=True)
                nc.tensor.matmul(Pc1b[:], lhsT=lhsT,
                                 rhs=rhs[:, 512:1024], start=True, stop=True)
                ev1 = ev_pool.tile([T, QK], F32, name="ev")
                nc.scalar.copy(ev1[:, 0:512], Pc1a[:])
                nc.scalar.copy(ev1[:, 512:1024], Pc1b[:])
                # direct-half 1: codes 2048..3071
                nc.tensor.matmul(Pd1[:, 0:512], lhsT=lhsT,
                                 rhs=rhs[:, 2048:2560], start=True, stop=True)
                nc.tensor.matmul(Pd1[:, 512:1024], lhsT=lhsT,
                                 rhs=rhs[:, 2560:3072], start=True, stop=True)
                nc.vector._custom_dve(
                    amax_op, out=scr[:], in0=Pd1[:], in1=ev1[:], s0=2.0,
                )
                # copy-half 2: codes 1024..2047
                nc.tensor.matmul(Pc2[:, 0:512], lhsT=lhsT,
                                 rhs=rhs[:, 1024:1536], start=True, stop=True)
                nc.tensor.matmul(Pc2[:, 512:1024], lhsT=lhsT,
                                 rhs=rhs[:, 1536:2048], start=True, stop=True)
                ev2 = ev_pool.tile([T, QK], F32, name="ev")
                nc.scalar.copy(ev2[:], Pc2[:])
                # direct-half 2: codes 3072..4095
                nc.tensor.matmul(Pd2[:, 0:512], lhsT=lhsT,
                                 rhs=rhs[:, 3072:3584], start=True, stop=True)
                nc.tensor.matmul(Pd2[:, 512:1024], lhsT=lhsT,
                                 rhs=rhs[:, 3584:4096], start=True, stop=True)
                nc.vector._custom_dve(
                    amax_cont_op, out=scr[:], in0=Pd2[:], in1=ev2[:], s0=2.0,
                    accum_out=idxf[:, cb:cb + 1],
                )
                # batched u32 cast + value gathers
                if cb % GB == GB - 1:
                    c0 = cb - GB + 1
                    nc.gpsimd.dma_start(idx_u[:, c0:cb + 1],
                                        idxf[:, c0:cb + 1])
                    for cg in range(c0, cb + 1):
                        nc.gpsimd.indirect_dma_start(
                            out=g[:, cg],
                            out_offset=None,
                            in_=vals.ap(),
                            in_offset=bass.IndirectOffsetOnAxis(
                                ap=idx_u[:, cg:cg + 1], axis=0
                            ),
                            element_offset=cg * K * d,
                            bounds_check=K - 1,
                            oob_is_err=False,
                        )

            # staircase prefetch: keep >=2 blocks in flight ahead of compute
            nc.sync.dma_start(x_sb[:], xT.ap())
            load_kT(0)
            load_kT(1)
            for i in range(CBC):
                if i % CB_PER_DMA == 0:
                    nxt = i // CB_PER_DMA + 2
                    if nxt < NDMA:
                        load_kT(nxt)
                do_cb(i)
                if i == CBC - 9:
                    # early partial store: first 24 codebooks' gathers done
                    nc.sync.dma_start(
                        out.ap()[:, 0:(CBC - 8) * d],
                        g[:, 0:CBC - 8].rearrange("t c dd -> t (c dd)"),
                    )

            nc.sync.dma_start(
                out.ap()[:, (CBC - 8) * d:],
                g[:, CBC - 8:].rearrange("t c dd -> t (c dd)"),
            )

    nc.compile()
    return nc


def _bf16_split(a: np.ndarray):
    """Dekker split: a ~= hi + lo with hi, lo exactly representable in bf16."""
    hi = a.astype(ml_dtypes.bfloat16)
    lo = (a - hi.astype(np.float32)).astype(ml_dtypes.bfloat16)
    return hi, lo


def make_core_inputs(embeddings: np.ndarray, keys: np.ndarray, values: np.ndarray):
    """Host-side shard prep. Returns list of input dicts, one per core."""
    NDMA = CBC // CB_PER_DMA
    # tokens: [B, D, H, W] -> [B*N, C, d]
    x = embeddings.reshape(B, D, H * W).transpose(0, 2, 1).reshape(T, C, d)
    xh, xl = _bf16_split(np.ascontiguousarray(x))
    # lhsT rows: [xh(8); 1; xh(8); 1; xl(8); xl(8)] -> [C, 34, T]
    rows = np.empty((C, KA, T), dtype=ml_dtypes.bfloat16)
    rows[:, 0:8] = xh.transpose(1, 2, 0)
    rows[:, 8] = 1.0
    rows[:, 9:17] = rows[:, 0:8]
    rows[:, 17] = 1.0
    rows[:, 18:26] = xl.transpose(1, 2, 0)
    rows[:, 26:34] = rows[:, 18:26]

    kh, kl = _bf16_split(keys)
    s = 0.5 * np.einsum("ckd,ckd->ck", keys, keys)
    sh, sl = _bf16_split(s)
    keysT = np.empty((C, KA, K), dtype=ml_dtypes.bfloat16)
    keysT[:, 0:8] = kh.transpose(0, 2, 1)
    keysT[:, 8] = -sh
    keysT[:, 9:17] = kl.transpose(0, 2, 1)
    keysT[:, 17] = -sl
    keysT[:, 18:26] = keysT[:, 0:8]
    keysT[:, 26:34] = keysT[:, 9:17]

    # values permuted so the encoded argmax v = 2*j + nb indexes directly:
    # v even -> code 2048 + v/2 (psum half), v odd -> code (v-1)/2 (evac half)
    v = np.arange(K)
    perm = np.where(v % 2 == 0, HK + v // 2, v // 2)
    vals_perm = values[:, perm, :]

    in_maps = []
    for i in range(NCORES):
        sl_ = slice(i * CBC, (i + 1) * CBC)
        rows_c = rows[sl_]                     # [CBC, 34, T]
        keysT_c = keysT[sl_]                   # [CBC, 34, K]

        xT = np.zeros((128, CBC // 2, T), dtype=ml_dtypes.bfloat16)
        xT[0:KA] = rows_c[0::2].transpose(1, 0, 2)
        xT[64:64 + KA] = rows_c[1::2].transpose(1, 0, 2)

        kt = np.zeros((NDMA, 128, 2, K), dtype=ml_dtypes.bfloat16)
        for blk in range(NDMA):
            for s2 in range(2):
                kt[blk, 0:KA, s2] = keysT_c[blk * 4 + 2 * s2]
                kt[blk, 64:64 + KA, s2] = keysT_c[blk * 4 + 2 * s2 + 1]

        in_maps.append({
            "xT": xT,
            "keysT": kt,
            "vals": np.ascontiguousarray(
                vals_perm[sl_].reshape(CBC * K, d).astype(np.float32)
            ),
        })
    return in_maps


def assemble_output(results: list) -> np.ndarray:
    """results[i]["out"] is [T, CBC*d] for core i; -> [B, D, H, W]."""
    mem = np.concatenate(
        [np.asarray(r["out"]).reshape(T, CBC * d) for r in results], axis=1
    )  # [T, C*d] == [B*N, D]
    return (
        mem.reshape(B, H * W, D).transpose(0, 2, 1).reshape(B, D, H, W)
    ).astype(np.float32)


_CACHED_NC = None


def kernel(embeddings, keys, values):
    global _CACHED_NC
    embeddings = np.asarray(embeddings, dtype=np.float32)
    keys = np.asarray(keys, dtype=np.float32)
    values = np.asarray(values, dtype=np.float32)
    if _CACHED_NC is None:
        _CACHED_NC = build_program()
    in_maps = make_core_inputs(embeddings, keys, values)
    res = run_bass_kernel_spmd(_CACHED_NC, in_maps, list(range(NCORES)))
    return assemble_output(res.results)


if __name__ == "__main__":
    rng = np.random.default_rng(0)
    emb = rng.standard_normal((B, D, H, W), dtype=np.float32)
    ks = rng.standard_normal((C, K, d), dtype=np.float32)
    vs = rng.standard_normal((C, K, d), dtype=np.float32)
    out = kernel(emb, ks, vs)
    print("out", out.shape, out.dtype, out.ravel()[:4])


# revision 11
# speedup vs baseline: 1.2412x; 1.1613x over previous
"""VQ codebook (DKVB) kernel for Trainium2, sharded over 8 NeuronCores.

Problem: embeddings [8, 2048, 4, 4] -> tokens x [128, 256, 8]; per codebook c
(256 of them), find nearest code among 4096 (euclidean), gather values row.

Strategy: shard the 256 codebooks across 8 cores (32 per core).
Score s[t,k] = x_t . k - |k|^2/2 (argmax s == argmin dist), computed as a
bf16 Dekker split folded into ONE matmul with contraction dim 34:
    lhsT rows = [xh(8); 1; xh(8); 1; xl(8); xl(8)]
    rhs  rows = [kh(8); -sh; kl(8); -sl; kh(8); kl(8)]

The PE runs at a fixed 512cols/427ns (mid p-state is the steady-state ceiling
on this stack), so the PE is the pacer: 8 matmuls/cb = 3.4us/cb = 109us.
Everything else is sized to hide under that:
  - Act: 2x copy [T,1024] PSUM->SBUF per cb (2.3us)
  - DVE: seed+continuation pair-scan, 2 scores/cycle (2.5us)
  - keysT DMA: codebooks are packed 2-per-free-range at partition offsets
    {0,64} (DMA cost on this stack is free-bytes-per-partition only), and
    loaded 4 codebooks per DMA on the SP queue (~50us total, hidden).

Argmax: custom two-source DVE op (ARGMAX2) pairs the PSUM stream (codes
2048+j, PSUM port) with the SBUF stream (codes j, evacuated by ScalarE):
    v = 2*j + (e_sbuf >= e_psum)
accum-max(v) is the encoded argmax; values are host-permuted so v indexes
them directly. The scan state lives in DVE stage flops which persist across
instructions, so the 2048-pair scan is seed (ARGMAX2) + continuation
(ARGMAX2C), letting PSUM quarters release mid-scan.
"""

import numpy as np
import ml_dtypes

import concourse.bass as bass
import concourse.tile as tile
from concourse import bacc, mybir
from concourse.bass_utils import run_bass_kernel_spmd

B, D, H, W = 8, 2048, 4, 4
C, K, d = 256, 4096, 8
NCORES = 8
CBC = C // NCORES          # 32 codebooks per core
T = B * H * W              # 128 tokens
KA = 34                    # Dekker-augmented contraction dim
HK = K // 2                # 2048
QK = K // 4                # 1024

F32 = mybir.dt.float32
BF16 = mybir.dt.bfloat16
U32 = mybir.dt.uint32

_FLT_MAX = np.float32(3.4028235e38)

CB_PER_DMA = 4             # 2 free-slots x {0,64} partition offsets


def _build_argmax2_spec():
    from concourse.dve_spec import (
        Spec, Src0, Src1, MaxNeg, Zero, C0, AluOp, Bin, Scan, select, maxx,
        scan,
    )

    pm = Bin(AluOp.MAX, Src0, Src1)
    nb = Bin(AluOp.IS_LE, Src0, Src1)      # 1 -> Src1 (sbuf, low codes) wins
    idx2 = Scan(AluOp.ADD, C0, init=Bin(AluOp.SUBTRACT, Zero, C0))  # 0,2,4,..
    v = Bin(AluOp.ADD, idx2, nb)
    g = scan(AluOp.MAX, pm)
    t = Bin(AluOp.IS_GE, pm, g)
    body = select(t, v, MaxNeg)

    def _ref(in0, in1, s0, s1, imm2):
        e0 = in0.astype(np.float32)
        e1 = in1.astype(np.float32)
        pm = np.maximum(e0, e1)
        nb = (e0 <= e1).astype(np.float32)
        r = np.maximum.accumulate(pm, axis=-1)
        j = np.broadcast_to(
            np.arange(pm.shape[-1], dtype=np.float32) * np.float32(s0), pm.shape
        )
        vv = j + nb
        bodyv = np.where(pm >= r, vv, -_FLT_MAX).astype(np.float32)
        acc = bodyv.reshape(bodyv.shape[0], -1).max(axis=-1, keepdims=True)
        return bodyv, acc

    return Spec(body=body, accum=maxx, reference=_ref)


def _register_ops():
    """Register ARGMAX2 (seeded) and ARGMAX2C (continuation). Idempotent."""
    from concourse import dve_ops
    from concourse.dve_spec import lower
    from concourse.dve_uop import DveOpSpec
    from dataclasses import dataclass

    have = {op.name: op for op in dve_ops.OPS}
    if "ARGMAX2_ANT" in have and "ARGMAX2C_ANT" in have:
        return have["ARGMAX2_ANT"], have["ARGMAX2C_ANT"]

    spec = _build_argmax2_spec()

    @dataclass(frozen=True)
    class ContDveOp(dve_ops.DveOp):
        """Continuation variant: steady-state uop only — inherits the scan /
        counter / accumulator state left by the previous DVE instruction."""

        def compile(self, ver):
            key = (self.name, ver)
            cached = dve_ops._COMPILE_CACHE.get(key)
            if cached is not None:
                return cached
            full = lower(self.spec, ver=ver)
            result = DveOpSpec(
                name=self.name,
                opcode=dve_ops.get_dve_sub_opcode(self.name),
                uops=[full[-1]],
                rd1_en=True,
            )
            got = result.sha(ver)
            if self.uops_sha.get(ver) != got:
                raise ValueError(f"{self.name}: sha drift {got}")
            dve_ops._COMPILE_CACHE[key] = result
            return result

    ops = []
    for name, cls, pick in (
        ("ARGMAX2_ANT", dve_ops.DveOp, None),
        ("ARGMAX2C_ANT", ContDveOp, -1),
    ):
        opcode = dve_ops._CUSTOM_DVE_ROW_BASE + len(dve_ops.OPS)
        shas = {}
        for ver in ("v3", "v4"):
            uops = lower(spec, ver=ver)
            if pick is not None:
                uops = [uops[pick]]
            s = DveOpSpec(name=name, opcode=opcode, uops=uops, rd1_en=True)
            shas[ver] = s.sha(ver)
        op = cls(name, spec, subdim=False, uops_sha=shas)
        dve_ops.OPS.append(op)
        dve_ops._SUB_OPCODE_FOR_NAME[op.name] = opcode
        dve_ops.CUSTOM_DVE_SPECS[op.name] = spec
        ops.append(op)
    return ops[0], ops[1]


def build_program():
    amax_op, amax_cont_op = _register_ops()
    nc = bacc.Bacc(trn_type="TRN2", num_devices=NCORES)

    NDMA = CBC // CB_PER_DMA   # 8 keysT blocks of 4 codebooks each

    # x rows duplicated at partitions 0-33 and 64-97; slot = cb//2
    xT = nc.dram_tensor("xT", [128, CBC // 2, T], BF16, kind="ExternalInput")
    # keysT packed: dma block i holds cb 4i..4i+3 as [128, 2, 4096]
    # (partitions 0-33 = even cb of the pair, 64-97 = odd cb)
    keysT = nc.dram_tensor("keysT", [NDMA, 128, 2, K], BF16,
                           kind="ExternalInput")
    vals = nc.dram_tensor("vals", [CBC * K, d], F32, kind="ExternalInput")
    out = nc.dram_tensor("out", [T, CBC * d], F32, kind="ExternalOutput")
    GB = 2                     # gather/cast batch (codebooks)

    with tile.TileContext(nc) as tc:
        with (
            tc.tile_pool(name="xsb", bufs=1) as x_pool,
            tc.tile_pool(name="kT", bufs=3) as kT_pool,
            tc.tile_pool(name="evac", bufs=4) as ev_pool,
            tc.tile_pool(name="scratch", bufs=1) as scr_pool,
            tc.tile_pool(name="persist", bufs=1) as persist_pool,
            tc.tile_pool(name="psum", bufs=1, space="PSUM") as psum_pool,
        ):
            x_sb = x_pool.tile([128, CBC // 2, T], BF16)

            idxf = persist_pool.tile([T, CBC], F32)
            idx_u = persist_pool.tile([T, CBC], U32)
            g = persist_pool.tile([T, CBC, d], F32)
            # single scratch shared by every ARGMAX2 op: the WAW chain forces
            # the vector engine to execute all pair-scans in emission order,
            # so nothing can slip between a seed op and its continuation
            scr = scr_pool.tile([T, QK], F32)

            # PSUM: copy half 1 split into two 1-bank tiles (shortens the
            # copy1 -> next-cb-matmul WAR arc, the critical cycle)
            Pc1a = psum_pool.tile([T, 512], F32, name="pc1a")
            Pc1b = psum_pool.tile([T, 512], F32, name="pc1b")
            Pc2 = psum_pool.tile([T, QK], F32, name="pc2")
            Pd1 = psum_pool.tile([T, QK], F32, name="pd1")
            Pd2 = psum_pool.tile([T, QK], F32, name="pd2")

            kTs = [None] * NDMA

            def load_kT(i):
                kT = kT_pool.tile([128, 2, K], BF16, name="kt")
                kTs[i] = kT
                if i == 0:
                    # staircase: slot pieces so cb0/1 are ready ASAP
                    nc.sync.dma_start(kT[:, 0], keysT.ap()[i, :, 0])
                    nc.sync.dma_start(kT[:, 1], keysT.ap()[i, :, 1])
                else:
                    nc.sync.dma_start(kT[:], keysT.ap()[i])

            def do_cb(cb):
                kT = kTs[cb // CB_PER_DMA]
                slot = (cb % CB_PER_DMA) // 2
                base = 64 * (cb % 2)
                lhsT = x_sb[base:base + KA, cb // 2]
                rhs = kT[base:base + KA, slot]

                # copy-half 1: codes 0..1023 (two 1-bank tiles)
                nc.tensor.matmul(Pc1a[:], lhsT=lhsT, rhs=rhs[:, 0:512],
                                 start=True, stop=True)
                nc.tensor.matmul(Pc1b[:], lhsT=lhsT,
                                 rhs=rhs[:, 512:1024], start=True, stop=True)
                ev1 = ev_pool.tile([T, QK], F32, name="ev")
                nc.scalar.copy(ev1[:, 0:512], Pc1a[:])
                nc.scalar.copy(ev1[:, 512:1024], Pc1b[:])
                # direct-half 1: codes 2048..3071
                nc.tensor.matmul(Pd1[:, 0:512], lhsT=lhsT,
                                 rhs=rhs[:, 2048:2560], start=True, stop=True)
                nc.tensor.matmul(Pd1[:, 512:1024], lhsT=lhsT,
                                 rhs=rhs[:, 2560:3072], start=True, stop=True)
                nc.vector._custom_dve(
                    amax_op, out=scr[:], in0=Pd1[:], in1=ev1[:], s0=2.0,
                )
                # copy-half 2: codes 1024..2047
                nc.tensor.matmul(Pc2[:, 0:512], lhsT=lhsT,
                                 rhs=rhs[:, 1024:1536], start=True, stop=True)
                nc.tensor.matmul(Pc2[:, 512:1024], lhsT=lhsT,
                                 rhs=rhs[:, 1536:2048], start=True, stop=True)
                ev2 = ev_pool.tile([T, QK], F32, name="ev")
                nc.scalar.copy(ev2[:], Pc2[:])
                # direct-half 2: codes 3072..4095
                nc.tensor.matmul(Pd2[:, 0:512], lhsT=lhsT,
                                 rhs=rhs[:, 3072:3584], start=True, stop=True)
                nc.tensor.matmul(Pd2[:, 512:1024], lhsT=lhsT,
                                 rhs=rhs[:, 3584:4096], start=True, stop=True)
                nc.vector._custom_dve(
                    amax_cont_op, out=scr[:], in0=Pd2[:], in1=ev2[:], s0=2.0,
                    accum_out=idxf[:, cb:cb + 1],
                )
                # u32 cast on the idle SP queue; per-cb gathers on gpsimd
                if cb % GB == GB - 1:
                    c0 = cb - GB + 1
                    nc.vector.tensor_copy(idx_u[:, c0:cb + 1],
                                          idxf[:, c0:cb + 1])
                    for cg in range(c0, cb + 1):
                        nc.gpsimd.indirect_dma_start(
                            out=g[:, cg],
                            out_offset=None,
                            in_=vals.ap(),
                            in_offset=bass.IndirectOffsetOnAxis(
                                ap=idx_u[:, cg:cg + 1], axis=0
                            ),
                            element_offset=cg * K * d,
                            bounds_check=K - 1,
                            oob_is_err=False,
                        )

            # staircase prefetch: keep >=2 blocks in flight ahead of compute
            nc.sync.dma_start(x_sb[:], xT.ap())
            load_kT(0)
            load_kT(1)
            for i in range(CBC):
                if i % CB_PER_DMA == 0:
                    nxt = i // CB_PER_DMA + 2
                    if nxt < NDMA:
                        load_kT(nxt)
                do_cb(i)
                if i == CBC - 9:
                    # early partial store: first 24 codebooks' gathers done
                    nc.sync.dma_start(
                        out.ap()[:, 0:(CBC - 8) * d],
                        g[:, 0:CBC - 8].rearrange("t c dd -> t (c dd)"),
                    )

            nc.sync.dma_start(
                out.ap()[:, (CBC - 8) * d:],
                g[:, CBC - 8:].rearrange("t c dd -> t (c dd)"),
            )

    nc.compile()
    return nc


def _bf16_split(a: np.ndarray):
    """Dekker split: a ~= hi + lo with hi, lo exactly representable in bf16."""
    hi = a.astype(ml_dtypes.bfloat16)
    lo = (a - hi.astype(np.float32)).astype(ml_dtypes.bfloat16)
    return hi, lo


def make_core_inputs(embeddings: np.ndarray, keys: np.ndarray, values: np.ndarray):
    """Host-side shard prep. Returns list of input dicts, one per core."""
    NDMA = CBC // CB_PER_DMA
    # tokens: [B, D, H, W] -> [B*N, C, d]
    x = embeddings.reshape(B, D, H * W).transpose(0, 2, 1).reshape(T, C, d)
    xh, xl = _bf16_split(np.ascontiguousarray(x))
    # lhsT rows: [xh(8); 1; xh(8); 1; xl(8); xl(8)] -> [C, 34, T]
    rows = np.empty((C, KA, T), dtype=ml_dtypes.bfloat16)
    rows[:, 0:8] = xh.transpose(1, 2, 0)
    rows[:, 8] = 1.0
    rows[:, 9:17] = rows[:, 0:8]
    rows[:, 17] = 1.0
    rows[:, 18:26] = xl.transpose(1, 2, 0)
    rows[:, 26:34] = rows[:, 18:26]

    kh, kl = _bf16_split(keys)
    s = 0.5 * np.einsum("ckd,ckd->ck", keys, keys)
    sh, sl = _bf16_split(s)
    keysT = np.empty((C, KA, K), dtype=ml_dtypes.bfloat16)
    keysT[:, 0:8] = kh.transpose(0, 2, 1)
    keysT[:, 8] = -sh
    keysT[:, 9:17] = kl.transpose(0, 2, 1)
    keysT[:, 17] = -sl
    keysT[:, 18:26] = keysT[:, 0:8]
    keysT[:, 26:34] = keysT[:, 9:17]

    # values permuted so the encoded argmax v = 2*j + nb indexes directly:
    # v even -> code 2048 + v/2 (psum half), v odd -> code (v-1)/2 (evac half)
    v = np.arange(K)
    perm = np.where(v % 2 == 0, HK + v // 2, v // 2)
    vals_perm = values[:, perm, :]

    in_maps = []
    for i in range(NCORES):
        sl_ = slice(i * CBC, (i + 1) * CBC)
        rows_c = rows[sl_]                     # [CBC, 34, T]
        keysT_c = keysT[sl_]                   # [CBC, 34, K]

        xT = np.zeros((128, CBC // 2, T), dtype=ml_dtypes.bfloat16)
        xT[0:KA] = rows_c[0::2].transpose(1, 0, 2)
        xT[64:64 + KA] = rows_c[1::2].transpose(1, 0, 2)

        kt = np.zeros((NDMA, 128, 2, K), dtype=ml_dtypes.bfloat16)
        for blk in range(NDMA):
            for s2 in range(2):
                kt[blk, 0:KA, s2] = keysT_c[blk * 4 + 2 * s2]
                kt[blk, 64:64 + KA, s2] = keysT_c[blk * 4 + 2 * s2 + 1]

        in_maps.append({
            "xT": xT,
            "keysT": kt,
            "vals": np.ascontiguousarray(
                vals_perm[sl_].reshape(CBC * K, d).astype(np.float32)
            ),
        })
    return in_maps


def assemble_output(results: list) -> np.ndarray:
    """results[i]["out"] is [T, CBC*d] for core i; -> [B, D, H, W]."""
    mem = np.concatenate(
        [np.asarray(r["out"]).reshape(T, CBC * d) for r in results], axis=1
    )  # [T, C*d] == [B*N, D]
    return (
        mem.reshape(B, H * W, D).transpose(0, 2, 1).reshape(B, D, H, W)
    ).astype(np.float32)


_CACHED_NC = None


def kernel(embeddings, keys, values):
    global _CACHED_NC
    embeddings = np.asarray(embeddings, dtype=np.float32)
    keys = np.asarray(keys, dtype=np.float32)
    values = np.asarray(values, dtype=np.float32)
    if _CACHED_NC is None:
        _CACHED_NC = build_program()
    in_maps = make_core_inputs(embeddings, keys, values)
    res = run_bass_kernel_spmd(_CACHED_NC, in_maps, list(range(NCORES)))
    return assemble_output(res.results)


if __name__ == "__main__":
    rng = np.random.default_rng(0)
    emb = rng.standard_normal((B, D, H, W), dtype=np.float32)
    ks = rng.standard_normal((C, K, d), dtype=np.float32)
    vs = rng.standard_normal((C, K, d), dtype=np.float32)
    out = kernel(emb, ks, vs)
    print("out", out.shape, out.dtype, out.ravel()[:4])


# revision 12
# speedup vs baseline: 1.2459x; 1.0038x over previous
"""VQ codebook (DKVB) kernel for Trainium2, sharded over 8 NeuronCores.

Problem: embeddings [8, 2048, 4, 4] -> tokens x [128, 256, 8]; per codebook c
(256 of them), find nearest code among 4096 (euclidean), gather values row.

Strategy: shard the 256 codebooks across 8 cores (32 per core).
Score s[t,k] = x_t . k - |k|^2/2 (argmax s == argmin dist), computed as a
bf16 Dekker split folded into ONE matmul with contraction dim 34:
    lhsT rows = [xh(8); 1; xh(8); 1; xl(8); xl(8)]
    rhs  rows = [kh(8); -sh; kl(8); -sl; kh(8); kl(8)]

The PE runs at a fixed 512cols/427ns (mid p-state is the steady-state ceiling
on this stack), so the PE is the pacer: 8 matmuls/cb = 3.4us/cb = 109us.
Everything else is sized to hide under that:
  - Act: 2x copy [T,1024] PSUM->SBUF per cb (2.3us)
  - DVE: seed+continuation pair-scan, 2 scores/cycle (2.5us)
  - keysT DMA: codebooks are packed 2-per-free-range at partition offsets
    {0,64} (DMA cost on this stack is free-bytes-per-partition only), and
    loaded 4 codebooks per DMA on the SP queue (~50us total, hidden).

Argmax: custom two-source DVE op (ARGMAX2) pairs the PSUM stream (codes
2048+j, PSUM port) with the SBUF stream (codes j, evacuated by ScalarE):
    v = 2*j + (e_sbuf >= e_psum)
accum-max(v) is the encoded argmax; values are host-permuted so v indexes
them directly. The scan state lives in DVE stage flops which persist across
instructions, so the 2048-pair scan is seed (ARGMAX2) + continuation
(ARGMAX2C), letting PSUM quarters release mid-scan.
"""

import numpy as np
import ml_dtypes

import concourse.bass as bass
import concourse.tile as tile
from concourse import bacc, mybir
from concourse.bass_utils import run_bass_kernel_spmd

B, D, H, W = 8, 2048, 4, 4
C, K, d = 256, 4096, 8
NCORES = 8
CBC = C // NCORES          # 32 codebooks per core
T = B * H * W              # 128 tokens
KA = 34                    # Dekker-augmented contraction dim
HK = K // 2                # 2048
QK = K // 4                # 1024

F32 = mybir.dt.float32
BF16 = mybir.dt.bfloat16
U32 = mybir.dt.uint32

_FLT_MAX = np.float32(3.4028235e38)

CB_PER_DMA = 4             # 2 free-slots x {0,64} partition offsets


def _build_argmax2_spec():
    from concourse.dve_spec import (
        Spec, Src0, Src1, MaxNeg, Zero, C0, AluOp, Bin, Scan, select, maxx,
        scan,
    )

    pm = Bin(AluOp.MAX, Src0, Src1)
    nb = Bin(AluOp.IS_LE, Src0, Src1)      # 1 -> Src1 (sbuf, low codes) wins
    idx2 = Scan(AluOp.ADD, C0, init=Bin(AluOp.SUBTRACT, Zero, C0))  # 0,2,4,..
    v = Bin(AluOp.ADD, idx2, nb)
    g = scan(AluOp.MAX, pm)
    t = Bin(AluOp.IS_GE, pm, g)
    body = select(t, v, MaxNeg)

    def _ref(in0, in1, s0, s1, imm2):
        e0 = in0.astype(np.float32)
        e1 = in1.astype(np.float32)
        pm = np.maximum(e0, e1)
        nb = (e0 <= e1).astype(np.float32)
        r = np.maximum.accumulate(pm, axis=-1)
        j = np.broadcast_to(
            np.arange(pm.shape[-1], dtype=np.float32) * np.float32(s0), pm.shape
        )
        vv = j + nb
        bodyv = np.where(pm >= r, vv, -_FLT_MAX).astype(np.float32)
        acc = bodyv.reshape(bodyv.shape[0], -1).max(axis=-1, keepdims=True)
        return bodyv, acc

    return Spec(body=body, accum=maxx, reference=_ref)


def _register_ops():
    """Register ARGMAX2 (seeded) and ARGMAX2C (continuation). Idempotent."""
    from concourse import dve_ops
    from concourse.dve_spec import lower
    from concourse.dve_uop import DveOpSpec
    from dataclasses import dataclass

    have = {op.name: op for op in dve_ops.OPS}
    if "ARGMAX2_ANT" in have and "ARGMAX2C_ANT" in have:
        return have["ARGMAX2_ANT"], have["ARGMAX2C_ANT"]

    spec = _build_argmax2_spec()

    @dataclass(frozen=True)
    class ContDveOp(dve_ops.DveOp):
        """Continuation variant: steady-state uop only — inherits the scan /
        counter / accumulator state left by the previous DVE instruction."""

        def compile(self, ver):
            key = (self.name, ver)
            cached = dve_ops._COMPILE_CACHE.get(key)
            if cached is not None:
                return cached
            full = lower(self.spec, ver=ver)
            result = DveOpSpec(
                name=self.name,
                opcode=dve_ops.get_dve_sub_opcode(self.name),
                uops=[full[-1]],
                rd1_en=True,
            )
            got = result.sha(ver)
            if self.uops_sha.get(ver) != got:
                raise ValueError(f"{self.name}: sha drift {got}")
            dve_ops._COMPILE_CACHE[key] = result
            return result

    ops = []
    for name, cls, pick in (
        ("ARGMAX2_ANT", dve_ops.DveOp, None),
        ("ARGMAX2C_ANT", ContDveOp, -1),
    ):
        opcode = dve_ops._CUSTOM_DVE_ROW_BASE + len(dve_ops.OPS)
        shas = {}
        for ver in ("v3", "v4"):
            uops = lower(spec, ver=ver)
            if pick is not None:
                uops = [uops[pick]]
            s = DveOpSpec(name=name, opcode=opcode, uops=uops, rd1_en=True)
            shas[ver] = s.sha(ver)
        op = cls(name, spec, subdim=False, uops_sha=shas)
        dve_ops.OPS.append(op)
        dve_ops._SUB_OPCODE_FOR_NAME[op.name] = opcode
        dve_ops.CUSTOM_DVE_SPECS[op.name] = spec
        ops.append(op)
    return ops[0], ops[1]


def build_program():
    amax_op, amax_cont_op = _register_ops()
    nc = bacc.Bacc(trn_type="TRN2", num_devices=NCORES)

    NDMA = CBC // CB_PER_DMA   # 8 keysT blocks of 4 codebooks each

    # x rows duplicated at partitions 0-33 and 64-97; slot = cb//2
    xT = nc.dram_tensor("xT", [128, CBC // 2, T], BF16, kind="ExternalInput")
    # keysT packed: dma block i holds cb 4i..4i+3 as [128, 2, 4096]
    # (partitions 0-33 = even cb of the pair, 64-97 = odd cb)
    keysT = nc.dram_tensor("keysT", [NDMA, 128, 2, K], BF16,
                           kind="ExternalInput")
    vals = nc.dram_tensor("vals", [CBC * K, d], F32, kind="ExternalInput")
    out = nc.dram_tensor("out", [T, CBC * d], F32, kind="ExternalOutput")
    GB = 2                     # gather/cast batch (codebooks)

    with tile.TileContext(nc) as tc:
        with (
            tc.tile_pool(name="xsb", bufs=1) as x_pool,
            tc.tile_pool(name="kT", bufs=3) as kT_pool,
            tc.tile_pool(name="evac", bufs=6) as ev_pool,
            tc.tile_pool(name="scratch", bufs=1) as scr_pool,
            tc.tile_pool(name="persist", bufs=1) as persist_pool,
            tc.tile_pool(name="psum", bufs=1, space="PSUM") as psum_pool,
        ):
            x_sb = x_pool.tile([128, CBC // 2, T], BF16)

            idxf = persist_pool.tile([T, CBC], F32)
            idx_u = persist_pool.tile([T, CBC], U32)
            g = persist_pool.tile([T, CBC, d], F32)
            # single scratch shared by every ARGMAX2 op: the WAW chain forces
            # the vector engine to execute all pair-scans in emission order,
            # so nothing can slip between a seed op and its continuation
            scr = scr_pool.tile([T, QK], F32)

            # PSUM: copy half 1 split into two 1-bank tiles (shortens the
            # copy1 -> next-cb-matmul WAR arc, the critical cycle)
            Pc1a = psum_pool.tile([T, 512], F32, name="pc1a")
            Pc1b = psum_pool.tile([T, 512], F32, name="pc1b")
            Pc2a = psum_pool.tile([T, 512], F32, name="pc2a")
            Pc2b = psum_pool.tile([T, 512], F32, name="pc2b")
            Pd1 = psum_pool.tile([T, QK], F32, name="pd1")
            Pd2 = psum_pool.tile([T, QK], F32, name="pd2")

            kTs = [None] * NDMA

            def load_kT(i):
                kT = kT_pool.tile([128, 2, K], BF16, name="kt")
                kTs[i] = kT
                if i == 0:
                    # staircase: slot pieces so cb0/1 are ready ASAP
                    nc.sync.dma_start(kT[:, 0], keysT.ap()[i, :, 0])
                    nc.sync.dma_start(kT[:, 1], keysT.ap()[i, :, 1])
                else:
                    nc.sync.dma_start(kT[:], keysT.ap()[i])

            def do_cb(cb):
                kT = kTs[cb // CB_PER_DMA]
                slot = (cb % CB_PER_DMA) // 2
                base = 64 * (cb % 2)
                lhsT = x_sb[base:base + KA, cb // 2]
                rhs = kT[base:base + KA, slot]

                # copy-half 1: codes 0..1023 (two 1-bank tiles)
                nc.tensor.matmul(Pc1a[:], lhsT=lhsT, rhs=rhs[:, 0:512],
                                 start=True, stop=True)
                nc.tensor.matmul(Pc1b[:], lhsT=lhsT,
                                 rhs=rhs[:, 512:1024], start=True, stop=True)
                ev1 = ev_pool.tile([T, QK], F32, name="ev")
                nc.scalar.copy(ev1[:, 0:512], Pc1a[:])
                nc.scalar.copy(ev1[:, 512:1024], Pc1b[:])
                # direct-half 1: codes 2048..3071
                nc.tensor.matmul(Pd1[:, 0:512], lhsT=lhsT,
                                 rhs=rhs[:, 2048:2560], start=True, stop=True)
                nc.tensor.matmul(Pd1[:, 512:1024], lhsT=lhsT,
                                 rhs=rhs[:, 2560:3072], start=True, stop=True)
                nc.vector._custom_dve(
                    amax_op, out=scr[:], in0=Pd1[:], in1=ev1[:], s0=2.0,
                )
                # copy-half 2: codes 1024..2047 (two 1-bank tiles)
                nc.tensor.matmul(Pc2a[:], lhsT=lhsT,
                                 rhs=rhs[:, 1024:1536], start=True, stop=True)
                ev2 = ev_pool.tile([T, QK], F32, name="ev")
                nc.scalar.copy(ev2[:, 0:512], Pc2a[:])
                nc.tensor.matmul(Pc2b[:], lhsT=lhsT,
                                 rhs=rhs[:, 1536:2048], start=True, stop=True)
                nc.scalar.copy(ev2[:, 512:1024], Pc2b[:])
                # direct-half 2: codes 3072..4095
                nc.tensor.matmul(Pd2[:, 0:512], lhsT=lhsT,
                                 rhs=rhs[:, 3072:3584], start=True, stop=True)
                nc.tensor.matmul(Pd2[:, 512:1024], lhsT=lhsT,
                                 rhs=rhs[:, 3584:4096], start=True, stop=True)
                nc.vector._custom_dve(
                    amax_cont_op, out=scr[:], in0=Pd2[:], in1=ev2[:], s0=2.0,
                    accum_out=idxf[:, cb:cb + 1],
                )
                # u32 cast on the idle SP queue; per-cb gathers on gpsimd
                if cb % GB == GB - 1:
                    c0 = cb - GB + 1
                    nc.gpsimd.tensor_copy(idx_u[:, c0:cb + 1],
                                          idxf[:, c0:cb + 1])
                    for cg in range(c0, cb + 1):
                        nc.gpsimd.indirect_dma_start(
                            out=g[:, cg],
                            out_offset=None,
                            in_=vals.ap(),
                            in_offset=bass.IndirectOffsetOnAxis(
                                ap=idx_u[:, cg:cg + 1], axis=0
                            ),
                            element_offset=cg * K * d,
                            bounds_check=K - 1,
                            oob_is_err=False,
                        )

            # staircase prefetch: keep >=2 blocks in flight ahead of compute
            nc.sync.dma_start(x_sb[:], xT.ap())
            load_kT(0)
            load_kT(1)
            for i in range(CBC):
                if i % CB_PER_DMA == 0:
                    nxt = i // CB_PER_DMA + 2
                    if nxt < NDMA:
                        load_kT(nxt)
                do_cb(i)
                if i == CBC - 9:
                    # early partial store: first 24 codebooks' gathers done
                    nc.sync.dma_start(
                        out.ap()[:, 0:(CBC - 8) * d],
                        g[:, 0:CBC - 8].rearrange("t c dd -> t (c dd)"),
                    )

            nc.sync.dma_start(
                out.ap()[:, (CBC - 8) * d:],
                g[:, CBC - 8:].rearrange("t c dd -> t (c dd)"),
            )

    nc.compile()
    return nc


def _bf16_split(a: np.ndarray):
    """Dekker split: a ~= hi + lo with hi, lo exactly representable in bf16."""
    hi = a.astype(ml_dtypes.bfloat16)
    lo = (a - hi.astype(np.float32)).astype(ml_dtypes.bfloat16)
    return hi, lo


def make_core_inputs(embeddings: np.ndarray, keys: np.ndarray, values: np.ndarray):
    """Host-side shard prep. Returns list of input dicts, one per core."""
    NDMA = CBC // CB_PER_DMA
    # tokens: [B, D, H, W] -> [B*N, C, d]
    x = embeddings.reshape(B, D, H * W).transpose(0, 2, 1).reshape(T, C, d)
    xh, xl = _bf16_split(np.ascontiguousarray(x))
    # lhsT rows: [xh(8); 1; xh(8); 1; xl(8); xl(8)] -> [C, 34, T]
    rows = np.empty((C, KA, T), dtype=ml_dtypes.bfloat16)
    rows[:, 0:8] = xh.transpose(1, 2, 0)
    rows[:, 8] = 1.0
    rows[:, 9:17] = rows[:, 0:8]
    rows[:, 17] = 1.0
    rows[:, 18:26] = xl.transpose(1, 2, 0)
    rows[:, 26:34] = rows[:, 18:26]

    kh, kl = _bf16_split(keys)
    s = 0.5 * np.einsum("ckd,ckd->ck", keys, keys)
    sh, sl = _bf16_split(s)
    keysT = np.empty((C, KA, K), dtype=ml_dtypes.bfloat16)
    keysT[:, 0:8] = kh.transpose(0, 2, 1)
    keysT[:, 8] = -sh
    keysT[:, 9:17] = kl.transpose(0, 2, 1)
    keysT[:, 17] = -sl
    keysT[:, 18:26] = keysT[:, 0:8]
    keysT[:, 26:34] = keysT[:, 9:17]

    # values permuted so the encoded argmax v = 2*j + nb indexes directly:
    # v even -> code 2048 + v/2 (psum half), v odd -> code (v-1)/2 (evac half)
    v = np.arange(K)
    perm = np.where(v % 2 == 0, HK + v // 2, v // 2)
    vals_perm = values[:, perm, :]

    in_maps = []
    for i in range(NCORES):
        sl_ = slice(i * CBC, (i + 1) * CBC)
        rows_c = rows[sl_]                     # [CBC, 34, T]
        keysT_c = keysT[sl_]                   # [CBC, 34, K]

        xT = np.zeros((128, CBC // 2, T), dtype=ml_dtypes.bfloat16)
        xT[0:KA] = rows_c[0::2].transpose(1, 0, 2)
        xT[64:64 + KA] = rows_c[1::2].transpose(1, 0, 2)

        kt = np.zeros((NDMA, 128, 2, K), dtype=ml_dtypes.bfloat16)
        for blk in range(NDMA):
            for s2 in range(2):
                kt[blk, 0:KA, s2] = keysT_c[blk * 4 + 2 * s2]
                kt[blk, 64:64 + KA, s2] = keysT_c[blk * 4 + 2 * s2 + 1]

        in_maps.append({
            "xT": xT,
            "keysT": kt,
            "vals": np.ascontiguousarray(
                vals_perm[sl_].reshape(CBC * K, d).astype(np.float32)
            ),
        })
    return in_maps


def assemble_output(results: list) -> np.ndarray:
    """results[i]["out"] is [T, CBC*d] for core i; -> [B, D, H, W]."""
    mem = np.concatenate(
        [np.asarray(r["out"]).reshape(T, CBC * d) for r in results], axis=1
    )  # [T, C*d] == [B*N, D]
    return (
        mem.reshape(B, H * W, D).transpose(0, 2, 1).reshape(B, D, H, W)
    ).astype(np.float32)


_CACHED_NC = None


def kernel(embeddings, keys, values):
    global _CACHED_NC
    embeddings = np.asarray(embeddings, dtype=np.float32)
    keys = np.asarray(keys, dtype=np.float32)
    values = np.asarray(values, dtype=np.float32)
    if _CACHED_NC is None:
        _CACHED_NC = build_program()
    in_maps = make_core_inputs(embeddings, keys, values)
    res = run_bass_kernel_spmd(_CACHED_NC, in_maps, list(range(NCORES)))
    return assemble_output(res.results)


if __name__ == "__main__":
    rng = np.random.default_rng(0)
    emb = rng.standard_normal((B, D, H, W), dtype=np.float32)
    ks = rng.standard_normal((C, K, d), dtype=np.float32)
    vs = rng.standard_normal((C, K, d), dtype=np.float32)
    out = kernel(emb, ks, vs)
    print("out", out.shape, out.dtype, out.ravel()[:4])
